# revision 46
# baseline (speedup 1.0000x reference)
"""MALA attention (linear attention w/ 2D RoPE + magnitude term) on 8 trn2 cores.

Sharding: core i handles batch b = i//2, sequence rows (i%2)*2048..+2048.
Cross-core data (kv = k_rope^T v, k_sum, v_sum -- all sums over the full
sequence) is combined with a pairwise AllReduce (~103KB bf16). Everything else
is local. bf16 operands everywhere with fp32 PSUM accumulation.

Math (per batch b, head h, reference semantics):
  q = elu(query @ Wq.T + bq) + 1      (same k with Wk/bk; v plain)
  z = (q . mean_n(k)) * d^-0.5
  q, k <- rope(q), rope(k)
  kv = k^T v * (d^-0.5 / N)
  res = (q @ kv) * (1 + 1/(z+1e-6)) - z * mean_n(v)
  out = res @ Wo.T + bo

Device-side formulation:
  elu(x)+1 = max(x + 1, min(exp(x), 1)) on the k path (DVE stt, bf16 ops);
  on the q path elu(u)+1 = min(exp(u),1) + relu(u): both exp and relu get
  the +bq bias via Act per-partition bias, so all q bias-opener matmuls are
  gone and the stt runs at bf16 rate.
  g = 1 + 1/(z+eps) folded into q_rope before o_proj (G built by one-hot
  matmul). kv is folded INTO the o_proj weights: W2 = (kv*SFAC) @ Wo.T per
  head (12 small matmuls post-CC, via a tensor-engine transpose of the kv
  block), so out = (q~g) @ W2 + z @ (-W~) + bo in a single accumulation --
  the separate qkv matmul and res round-trip are gone.
  -z*v_mean term folded into o_proj as an extra matmul (-W~)^T @ z where
  W~[c,h] = Wo[:, head h] @ v_mean_h.
  The within-head d-index is permuted (evens first) on Wq/Wk columns + trig
  tables so rotate_every_two becomes a 32-column block swap.
  xk/xv/xq are host-repacked chunk-major so every input DMA is 128 fat
  per-partition descriptors (no 256B slivers).
  Collective payload is bf16, pre-scaled by SFAC where possible: kv
  v-halves+vsum row [65,768] plus per-head ksums pre-transposed to columns
  [128,12]. Payload DMAs ride the otherwise-empty gpsimd queue right in
  front of the CC trigger; post-CC loads ride sync (parked at the CC wait),
  sums first; kblock assembly is a plain gpsimd copy so the z chain doesn't
  queue behind phase-2a activations.
"""

import os
import tempfile

import numpy as np
import ml_dtypes

NUM_HEADS = 12
B, N, C = 4, 4096, 768
D = 64
NCORES = 8
NLOC = N // 2          # rows per core
SCALE = D ** -0.5
BF16 = ml_dtypes.bfloat16

_CACHE = {}
LAST_RESULTS = [None]  # test.py reads profiling info from here


# --------------------------------------------------------------------------
# host-side helpers
# --------------------------------------------------------------------------

def _perm64():
    # evens first, then odds (within each head's 64 dims)
    return np.concatenate([np.arange(0, 64, 2), np.arange(1, 64, 2)])


def _trig_tables():
    """c32/s32: [N, 32] fp32, value of cos/sin at original dim 2i (== 2i+1)."""
    H = W = 64
    angle = 1.0 / (10000.0 ** np.linspace(0.0, 1.0, D // 4))
    angle = np.repeat(angle, 2)                          # [32]
    ih = np.arange(H, dtype=np.float64)[:, None] * angle[None, :]   # [H, 32]
    iw = np.arange(W, dtype=np.float64)[:, None] * angle[None, :]
    sin_h, cos_h = np.sin(ih), np.cos(ih)
    sin_w, cos_w = np.sin(iw), np.cos(iw)
    r = np.arange(N) // W
    c = np.arange(N) % W
    s_full = np.concatenate([sin_h[r], sin_w[c]], axis=1)   # [N, 64]
    c_full = np.concatenate([cos_h[r], cos_w[c]], axis=1)
    c32 = c_full[:, 0::2].astype(np.float32)
    s32 = s_full[:, 0::2].astype(np.float32)
    return c32, s32


def _chunk_pack(xT, nchunk, nfree):
    # [C, NLOC] -> [nchunk, 128, 6, nfree]: one fat descriptor per partition
    a = xT.reshape(6, 128, nchunk, nfree)
    return np.ascontiguousarray(a.transpose(2, 1, 0, 3)).astype(BF16)


def _build_host_inputs(query, key, value, Wq, bq, Wk, bk, Wv, bv, Wo, bo):
    p64 = _perm64()
    perm = (np.arange(NUM_HEADS)[:, None] * 64 + p64[None, :]).reshape(-1)

    wq = np.ascontiguousarray(Wq.T[:, perm]).astype(BF16)
    wk = np.ascontiguousarray(Wk.T[:, perm]).astype(BF16)
    wv = np.ascontiguousarray(Wv.T).astype(BF16)
    wo = np.ascontiguousarray(Wo.T).astype(BF16)
    # q bias as per-partition columns for the Act bias add: [128, 6]
    bq_col = np.ascontiguousarray(
        bq[perm].astype(np.float32).reshape(6, 128).T)
    # k/v biases pre-broadcast across partitions for the DVE bias add
    bkp1_b = np.tile((bk[perm] + 1.0).astype(BF16)[None, :], (128, 1))
    bv_b = np.tile(bv.astype(BF16)[None, :], (128, 1))
    bof = bo.astype(np.float32)
    # identity for the kv transpose (values unused by the transpose datapath)
    ident = np.eye(64).astype(BF16)

    c32, s32 = _trig_tables()
    halves = []
    for hi in range(2):
        sl = slice(hi * NLOC, (hi + 1) * NLOC)
        ck = np.concatenate([c32[sl], c32[sl]], axis=1).astype(BF16)    # [NLOC, 64]
        s2k = np.concatenate([-s32[sl], s32[sl]], axis=1).astype(BF16)
        cq = np.tile(c32[sl].T, (4, 1)).astype(BF16)                    # [128, NLOC]
        s2q = np.tile(np.concatenate([-s32[sl].T, s32[sl].T], 0), (2, 1)).astype(BF16)
        halves.append((ck, s2k, cq, s2q))

    # one-hot G-broadcast lhsT: eblk[c][h, p] = 1 iff head h owns partition p
    # of chunk c (heads 2c: p<64, 2c+1: p>=64)
    eblk = np.zeros((6, NUM_HEADS, 128), dtype=BF16)
    for cc in range(6):
        eblk[cc, 2 * cc, :64] = 1.0
        eblk[cc, 2 * cc + 1, 64:] = 1.0

    in_maps = []
    for core in range(NCORES):
        b = core // 2
        hi = core % 2
        sl = slice(hi * NLOC, (hi + 1) * NLOC)
        ck, s2k, cq, s2q = halves[hi]
        in_maps.append({
            "xq": np.ascontiguousarray(query[b, sl].T).astype(BF16),
            "xk": np.ascontiguousarray(key[b, sl].T).astype(BF16),
            "xv": np.ascontiguousarray(value[b, sl].T).astype(BF16),
            "wq": wq, "wk": wk, "wv": wv, "wo": wo,
            "bq_col": bq_col, "bkp1_b": bkp1_b, "bv_b": bv_b, "bo": bof,
            "cos_k": ck, "s2_k": s2k, "cos_q": cq, "s2_q": s2q,
            "eblk": eblk, "ident": ident,
        })
    return in_maps


# --------------------------------------------------------------------------
# device kernel
# --------------------------------------------------------------------------

def _build_nc():
    import concourse.bacc as bacc
    import concourse.mybir as mybir
    import concourse.tile as tile

    fp32 = mybir.dt.float32
    bf16 = mybir.dt.bfloat16
    AF = mybir.ActivationFunctionType
    OP = mybir.AluOpType

    nc = bacc.Bacc("TRN2", target_bir_lowering=False, debug=False,
                   num_devices=NCORES)

    def din(name, shape, dt=bf16):
        return nc.dram_tensor(name, shape, dt, kind="ExternalInput").ap()

    NF = NLOC // 512       # 4 n-slices of 512
    NK = NLOC // 128       # 16 chunks of 128 for k/v phase

    xq = din("xq", [C, NLOC])
    xk = din("xk", [C, NLOC])
    xv = din("xv", [C, NLOC])
    wq_d = din("wq", [C, C])
    wk_d = din("wk", [C, C])
    wv_d = din("wv", [C, C])
    wo_d = din("wo", [C, C])
    bq_col_d = din("bq_col", [128, 6], fp32)
    bkp1_b_d = din("bkp1_b", [128, C])
    bv_b_d = din("bv_b", [128, C])
    bo_d = din("bo", [C], fp32)
    cos_k_d = din("cos_k", [NLOC, 64])
    s2_k_d = din("s2_k", [NLOC, 64])
    cos_q_d = din("cos_q", [128, NLOC])
    s2_q_d = din("s2_q", [128, NLOC])
    eblk_d = din("eblk", [6, NUM_HEADS, 128])
    ident_d = din("ident", [64, 64])
    fp16 = mybir.dt.float16
    outT = nc.dram_tensor("outT", [C, NLOC], fp16, kind="ExternalOutput").ap()
    DEBUG = os.environ.get("KERNEL_DEBUG") == "1"
    if DEBUG:
        dbg_qrope = nc.dram_tensor("dbg_qrope", [128, 6, NLOC], bf16, kind="ExternalOutput").ap()
        dbg_zb = nc.dram_tensor("dbg_zb", [NUM_HEADS, NLOC], bf16, kind="ExternalOutput").ap()
        dbg_sums = nc.dram_tensor("dbg_sums", [128, 12], bf16, kind="ExternalOutput").ap()
        dbg_w2 = nc.dram_tensor("dbg_w2", [128, 6, C], bf16, kind="ExternalOutput").ap()
        dbg_wneg = nc.dram_tensor("dbg_wneg", [NUM_HEADS, C], bf16, kind="ExternalOutput").ap()
        dbg_kblock = nc.dram_tensor("dbg_kblock", [128, 6, NUM_HEADS], bf16, kind="ExternalOutput").ap()

    SFAC = SCALE / N       # kv and z scale (2^-15, exact in bf16)
    NPAY = 65 * 768 + 128 * 12   # collective payload in bf16 elems

    with tile.TileContext(nc) as tc:
        from contextlib import ExitStack
        with ExitStack() as ctx:
            consts = ctx.enter_context(tc.tile_pool(name="consts", bufs=1))
            resid = ctx.enter_context(tc.tile_pool(name="resid", bufs=1))
            xin = ctx.enter_context(tc.tile_pool(name="xin", bufs=3))
            qx = ctx.enter_context(tc.tile_pool(name="qx", bufs=4))
            work = ctx.enter_context(tc.tile_pool(name="work", bufs=2))
            single = ctx.enter_context(tc.tile_pool(name="single", bufs=1))
            big = ctx.enter_context(tc.tile_pool(name="big", bufs=1))
            gsb = ctx.enter_context(tc.tile_pool(name="gsb", bufs=4))
            dram = ctx.enter_context(tc.tile_pool(name="dram", bufs=1, space="DRAM"))
            ph1 = ExitStack()
            pps = ph1.enter_context(tc.tile_pool(name="pps", bufs=4, space="PSUM"))
            kvp = ph1.enter_context(tc.tile_pool(name="kvp", bufs=1, space="PSUM"))

            # ---- phase-1-critical constants. xk0/xv0 ride the FRONT of the
            # scalar queue so they aren't stuck behind 2.4MB of weights on
            # the sync side ----
            xk0 = xin.tile([128, 6, 128], bf16, tag="xk_t", name="xk0")
            nc.scalar.dma_start(xk0[:], xk[:, 0:128].rearrange("(s p) n -> p s n", p=128))
            xv0 = xin.tile([128, 6, 128], bf16, tag="xv_t", name="xv0")
            nc.scalar.dma_start(xv0[:], xv[:, 0:128].rearrange("(s p) n -> p s n", p=128))
            wk_s = consts.tile([128, 6, C], bf16, tag="wk")
            for s in range(6):
                nc.sync.dma_start(wk_s[:, s, :],
                                  wk_d.rearrange("(s p) o -> p s o", p=128)[:, s, :])
            wv_s = consts.tile([128, 6, C], bf16, tag="wv")
            for s in range(6):
                nc.sync.dma_start(wv_s[:, s, :],
                                  wv_d.rearrange("(s p) o -> p s o", p=128)[:, s, :])
            bkb_s = consts.tile([128, C], bf16, tag="bkb")
            nc.scalar.dma_start(bkb_s[:], bkp1_b_d[:])
            bvb_s = consts.tile([128, C], bf16, tag="bvb")
            nc.scalar.dma_start(bvb_s[:], bv_b_d[:])
            cos_k_s = consts.tile([128, NK, 64], bf16, tag="cos_k")
            nc.scalar.dma_start(cos_k_s[:], cos_k_d.rearrange("(j p) d -> p j d", p=128))
            s2_k_s = consts.tile([128, NK, 64], bf16, tag="s2_k")
            nc.scalar.dma_start(s2_k_s[:], s2_k_d.rearrange("(j p) d -> p j d", p=128))
            xk1 = xin.tile([128, 6, 128], bf16, tag="xk_t", name="xk1")
            nc.sync.dma_start(xk1[:], xk[:, 128:256].rearrange("(s p) n -> p s n", p=128))
            xv1 = xin.tile([128, 6, 128], bf16, tag="xv_t", name="xv1")
            nc.sync.dma_start(xv1[:], xv[:, 128:256].rearrange("(s p) n -> p s n", p=128))
            bo_s = consts.tile([128, 6], fp32, tag="bo")
            nc.scalar.dma_start(bo_s[:], bo_d.rearrange("(s p) -> p s", p=128))
            bqc_s = consts.tile([128, 6], fp32, tag="bqc")
            nc.scalar.dma_start(bqc_s[:], bq_col_d[:])
            ident_s = consts.tile([64, 64], bf16, tag="ident")
            nc.scalar.dma_start(ident_s[:], ident_d[:])

            ones_row = consts.tile([1, 512], bf16, tag="ones_row")
            nc.vector.memset(ones_row[:], 1.0)
            negone = consts.tile([128, 1], fp32, tag="negone")
            nc.vector.memset(negone[:], -1.0)
            zero_col = consts.tile([1, 128], bf16, tag="zero_col")
            nc.vector.memset(zero_col[:], 0.0)
            # pre-scaled transpose constants: ksum cols arrive *SFAC,
            # vsum cols arrive *(-1/N) (both exact powers of two in bf16)
            sfac_b = consts.tile([1, 24], bf16, tag="sfac_b")
            nc.vector.memset(sfac_b[:], SFAC)
            negn_b = consts.tile([1, 24], bf16, tag="negn_b")
            nc.vector.memset(negn_b[:], -1.0 / N)

            # ---- persistent tiles ----
            qpre = big.tile([128, 6, NLOC], bf16, tag="qbig", name="qpre")
            qrope = resid.tile([128, 6, NLOC], bf16, tag="qrope")
            zb = resid.tile([NUM_HEADS, NLOC], bf16, tag="zb")
            kvT2 = resid.tile([128, 6, 128], bf16, tag="kvT2")
            nc.vector.memset(kvT2[:], 0.0)
            w2_s = consts.tile([128, 6, C], bf16, tag="w2")
            kblock = resid.tile([128, 6, NUM_HEADS], bf16, tag="kblock")
            nc.vector.memset(kblock[:], 0.0)
            vbneg = resid.tile([128, 6, NUM_HEADS], bf16, tag="vbneg")
            nc.vector.memset(vbneg[:], 0.0)
            wneg = resid.tile([NUM_HEADS, C], bf16, tag="wneg")

            # kv psums: 3 banks, persist through phase 1.
            # head h accumulates at [0:65, (h%4)*128 : +128] of tile h//4.
            # start=True clears the whole bank's has_written bits, so packing 4
            # heads' accumulation groups per bank needs a single bank-wide
            # zero-write group opener; all kv matmuls then accumulate.
            kvps = [kvp.tile([128, 512], fp32, tag=f"kvps{t}", name=f"kvps{t}")
                    for t in range(3)]
            for t in range(3):
                nc.tensor.matmul(kvps[t][0:65, :], zero_col[:, 0:65], ones_row[:],
                                 start=True, stop=False, skip_group_check=True)
            sums_ps = kvp.tile([128, 12], fp32, tag="sums_ps")

            # ================= phase 1: k/v proj, elu, rope, kv =================
            for j in range(NK):
                if j == 0:
                    xk_t, xv_t = xk0, xv0
                elif j == 1:
                    xk_t, xv_t = xk1, xv1
                else:
                    xk_t = xin.tile([128, 6, 128], bf16, tag="xk_t",
                                    name=f"xk{j}")
                    nc.sync.dma_start(
                        xk_t[:], xk[:, j * 128:(j + 1) * 128]
                        .rearrange("(s p) n -> p s n", p=128))
                    xv_t = xin.tile([128, 6, 128], bf16, tag="xv_t",
                                    name=f"xv{j}")
                    nc.sync.dma_start(
                        xv_t[:], xv[:, j * 128:(j + 1) * 128]
                        .rearrange("(s p) n -> p s n", p=128))

                vk = work.tile([128, NUM_HEADS, 128], bf16, tag="vk")
                e_t = work.tile([128, C], bf16, tag="e_t")
                tk = work.tile([128, C], bf16, tag="tk")
                kra = work.tile([128, NUM_HEADS, 66], bf16, tag="kra")
                nc.vector.memset(kra[:, :, 64:65], 1.0)

                # k projection (no bias opener; bias via DVE add)
                psk = [pps.tile([128, 384], fp32, tag="pp384", name=f"psk{half}")
                       for half in range(2)]
                for s in range(6):
                    for half in range(2):
                        nc.tensor.matmul(psk[half][:], xk_t[:, s, :],
                                         wk_s[:, s, half * 384:(half + 1) * 384],
                                         start=(s == 0), stop=(s == 5))
                for half in range(2):
                    osl = slice(half * 384, (half + 1) * 384)
                    hsl = slice(half * 6, (half + 1) * 6)
                    # tk = x + 1 (bias tile holds bk+1)
                    nc.vector.tensor_tensor(tk[:, osl], psk[half][:],
                                            bkb_s[:, osl], OP.add)
                    nc.scalar.activation(e_t[:, osl], tk[:, osl], AF.Exp,
                                         bias=negone[:])
                    nc.vector.scalar_tensor_tensor(
                        vk[:, hsl, 64:128],
                        e_t[:, osl].rearrange("p (h e) -> p h e", e=64),
                        1.0, tk[:, osl].rearrange("p (h e) -> p h e", e=64),
                        OP.min, OP.max)

                # v projection (bias via DVE add)
                psv = [pps.tile([128, 384], fp32, tag="pp384", name=f"psv{half}")
                       for half in range(2)]
                for s in range(6):
                    for half in range(2):
                        nc.tensor.matmul(psv[half][:], xv_t[:, s, :],
                                         wv_s[:, s, half * 384:(half + 1) * 384],
                                         start=(s == 0), stop=(s == 5))
                for half in range(2):
                    osl = slice(half * 384, (half + 1) * 384)
                    hsl = slice(half * 6, (half + 1) * 6)
                    nc.vector.tensor_tensor(
                        vk[:, hsl, 0:64],
                        psv[half][:].rearrange("p (h e) -> p h e", e=64),
                        bvb_s[:, osl].rearrange("p (h e) -> p h e", e=64),
                        OP.add)

                # rope on k_pre -> kra[:, :, 0:64]; s2 mults on gpsimd.
                # split per head-half so the kv matmuls of heads 0-5 fire
                # before heads 6-11's rope finishes (shorter chunk tail)
                tmpb = work.tile([128, NUM_HEADS, 64], bf16, tag="tmpb")
                for hh in range(2):
                    hsl2 = slice(hh * 6, (hh + 1) * 6)
                    cosj = cos_k_s[:, j, None, :].to_broadcast([128, 6, 64])
                    s2t = s2_k_s[:, j, None, 0:32].to_broadcast([128, 6, 32])
                    s2b = s2_k_s[:, j, None, 32:64].to_broadcast([128, 6, 32])
                    nc.vector.tensor_tensor(kra[:, hsl2, 0:64],
                                            vk[:, hsl2, 64:128], cosj, OP.mult)
                    nc.gpsimd.tensor_tensor(tmpb[:, hsl2, 0:32],
                                            vk[:, hsl2, 96:128], s2t, OP.mult)
                    nc.gpsimd.tensor_tensor(tmpb[:, hsl2, 32:64],
                                            vk[:, hsl2, 64:96], s2b, OP.mult)
                    nc.vector.tensor_tensor(kra[:, hsl2, 0:64],
                                            kra[:, hsl2, 0:64],
                                            tmpb[:, hsl2, :], OP.add)

                # kv accumulation: [k_rope | 1]^T @ [v | k_pre] per head
                for h in range(NUM_HEADS):
                    nc.tensor.matmul(
                        kvps[h // 4][0:65, (h % 4) * 128:(h % 4) * 128 + 128],
                        kra[:, h, 0:65], vk[:, h, :],
                        start=False, stop=(j == NK - 1), skip_group_check=True)

            # ====== phase 1.5: slim bf16 payload + pre-transposed ksums, CC
            # v-halves of kv psums (rows 0:65; row 64 = vsum) -> [65, 3, 256]
            # on DVE (Act handles the rows; parallel payload prep)
            kvsb_v = single.tile([65, 3, 256], bf16, tag="kvsb_v")
            for t in range(3):
                nc.vector.tensor_copy(
                    kvsb_v[:, t, :].rearrange("p (m e) -> p m e", e=64),
                    kvps[t][0:65, :].rearrange("p (m c) -> p m c", c=128)[:, :, 0:64])
            # full sums row (row 64: [vsum_h | ksum_h] per 128-col head block)
            kvs_row = single.tile([1, 1536], bf16, tag="kvs_row")
            for t in range(3):
                nc.scalar.activation(kvs_row[:, t * 512:(t + 1) * 512],
                                     kvps[t][64:65, :], AF.Copy)
            # transpose ksums to columns via bf16 K=1 matmuls (pre-scaled by
            # SFAC); col h = ksum_h*SFAC at partitions (h%2)*64 .. +64
            for h in range(NUM_HEADS):
                wk_off = 128 * h + 64 - 64 * (h % 2)
                nc.tensor.matmul(sums_ps[:, h:h + 1],
                                 kvs_row[:, wk_off:wk_off + 128],
                                 sfac_b[:, 0:1],
                                 start=True, stop=True, skip_group_check=True)
            sums_sb = single.tile([128, 12], bf16, tag="sums_sb")
            nc.scalar.activation(sums_sb[:], sums_ps[:], AF.Copy)

            # payload pushes on the gpsimd queue (empty, and the CC trigger
            # is the next gpsimd instruction -> no foreign descriptors ahead)
            bounce_in = dram.tile([NPAY], bf16, tag="b_in")
            bounce_out = dram.tile([NPAY], bf16, tag="b_out")
            nc.gpsimd.dma_start(
                bounce_in[0:65 * 768].rearrange("(p f) -> p f", p=65),
                kvsb_v.rearrange("p a b -> p (a b)"))
            nc.gpsimd.dma_start(
                bounce_in[65 * 768:NPAY].rearrange("(p f) -> p f", p=128),
                sums_sb[:])
            nc.gpsimd.collective_compute(
                "AllReduce", OP.add,
                replica_groups=[[0, 1], [2, 3], [4, 5], [6, 7]],
                ins=[bounce_in.opt()], outs=[bounce_out.opt()])

            # ---- late consts (needed from phase 2a on). xq slices ride in
            # front of / between the weights so slice-0 isn't stuck behind
            # 2.4MB of wq/wo ----
            xq_ts = [qx.tile([128, 6, 512], bf16, tag="xq_t", name=f"xq{nq}")
                     for nq in range(NF)]

            def xq_load(nq):
                nc.sync.dma_start(
                    xq_ts[nq][:], xq[:, nq * 512:(nq + 1) * 512]
                    .rearrange("(s p) n -> p s n", p=128))

            xq_load(0)
            wq_s = consts.tile([128, 6, C], bf16, tag="wq")
            nc.sync.dma_start(wq_s[:], wq_d.rearrange("(s p) o -> p s o", p=128))
            xq_load(1)
            cos_q_s = consts.tile([128, NLOC], bf16, tag="cos_q")
            nc.sync.dma_start(cos_q_s[:], cos_q_d[:])
            s2_q_s = consts.tile([128, NLOC], bf16, tag="s2_q")
            nc.sync.dma_start(s2_q_s[:], s2_q_d[:])
            for nq in range(2, NF):
                xq_load(nq)
            wo_s = consts.tile([128, 6, C], bf16, tag="wo")
            nc.sync.dma_start(wo_s[:], wo_d.rearrange("(s p) o -> p s o", p=128))
            eblk_s = consts.tile([NUM_HEADS, 6, 128], bf16, tag="eblk")
            nc.sync.dma_start(eblk_s[:], eblk_d.rearrange("c h p -> h c p"))

            # free phase-1 psum banks; phase 2a gets all 8 for psq
            ph1.close()
            ph2 = ExitStack()
            qps = ph2.enter_context(tc.tile_pool(name="qps", bufs=8, space="PSUM"))

            # ================= phase 2a: q proj, elu, rope =================
            for nq in range(NF):
                nsl = slice(nq * 512, (nq + 1) * 512)
                xq_t = xq_ts[nq]
                for oc in range(6):
                    psq = qps.tile([128, 512], fp32, tag="psq", name=f"psq{nq}_{oc}")
                    for s in range(6):
                        nc.tensor.matmul(psq[:], wq_s[:, s, oc * 128:(oc + 1) * 128],
                                         xq_t[:, s, :], start=(s == 0), stop=(s == 5))
                    # elu(u)+1 = min(exp(u),1) + relu(u); bias via Act bias ptr
                    e_q = work.tile([128, 512], bf16, tag="e_q")
                    nc.scalar.activation(e_q[:], psq[:], AF.Exp,
                                         bias=bqc_s[:, oc:oc + 1])
                    rx = work.tile([128, 512], bf16, tag="rx")
                    nc.scalar.activation(rx[:], psq[:], AF.Relu,
                                         bias=bqc_s[:, oc:oc + 1])
                    nc.vector.scalar_tensor_tensor(
                        qpre[:, oc, nsl], e_q[:], 1.0, rx[:], OP.min, OP.add)

                # rope: A + B with B reading the 32-block-swapped q_pre.
                # swap DMAs ride the gpsimd queue (sync is busy with consts)
                qsw = work.tile([128, 6, 512], bf16, tag="qsw")
                for g4 in range(4):
                    sp = (g4 ^ 1) * 32
                    nc.gpsimd.dma_start(qsw[g4 * 32:(g4 + 1) * 32, :, :],
                                        qpre[sp:sp + 32, :, nsl])
                for oc in range(6):
                    nc.vector.tensor_tensor(qrope[:, oc, nsl], qpre[:, oc, nsl],
                                            cos_q_s[:, nsl], OP.mult)
                    tmpq = work.tile([128, 512], bf16, tag="tmpq")
                    eng = nc.gpsimd if oc < 3 else nc.vector
                    eng.tensor_tensor(tmpq[:], qsw[:, oc, :], s2_q_s[:, nsl],
                                      OP.mult)
                    nc.vector.tensor_tensor(qrope[:, oc, nsl], qrope[:, oc, nsl],
                                            tmpq[:], OP.add)

            # ================= phase 2b: post-collective assembly =============
            # post-CC loads ride the SYNC queue: the sync engine has issued
            # everything else already and parks at the CC-wait. sums first --
            # they gate the z chain.
            sums_all = single.tile([128, 12], bf16, tag="sums_all")
            nc.sync.dma_start(sums_all[:],
                              bounce_out[65 * 768:NPAY].rearrange("(p f) -> p f", p=128))
            kvall_v = single.tile([65, 768], bf16, tag="kvall_v")
            nc.sync.dma_start(kvall_v[:],
                              bounce_out[0:65 * 768].rearrange("(p f) -> p f", p=65))

            # kblock (z weights; payload already *SFAC): strided SBUF-SBUF
            # DMAs on the parked sync queue -- no busy engine in the way
            kb_flat = kblock.rearrange("p s h -> p (s h)")
            vb_flat = vbneg.rearrange("p s h -> p (s h)")
            for t in range(2):
                psl = slice(t * 64, (t + 1) * 64)
                nc.sync.dma_start(kb_flat[psl, t:t + 71:14],
                                  sums_all[psl, t:12:2])

            ph2.close()
            zpool = ctx.enter_context(tc.tile_pool(name="zpool", bufs=2, space="PSUM"))
            gpool = ctx.enter_context(tc.tile_pool(name="gpool", bufs=2, space="PSUM"))
            w2p = ctx.enter_context(tc.tile_pool(name="w2p", bufs=2, space="PSUM"))
            opool = ctx.enter_context(tc.tile_pool(name="opool", bufs=2, space="PSUM"))

            # ========== phase 2c: z chains for all slices first ==========
            gbs = []
            for nq in range(NF):
                nsl = slice(nq * 512, (nq + 1) * 512)
                psz = zpool.tile([128, 512], fp32, tag="zp", name=f"psz{nq}")[0:NUM_HEADS, :]
                for s in range(6):
                    nc.tensor.matmul(psz[:], kblock[:, s, :], qpre[:, s, nsl],
                                     start=(s == 0), stop=(s == 5))
                gf = work.tile([NUM_HEADS, 512], fp32, tag="gf")
                nc.vector.reciprocal_approx_fast(gf[:], psz[:])  # z >= 7, eps moot
                gb = gsb.tile([NUM_HEADS, 512], bf16, tag="gb", name=f"gb{nq}")
                nc.vector.tensor_scalar_add(gb[:], gf[:], 1.0)
                nc.scalar.activation(zb[:, nsl], psz[:], AF.Copy)
                gbs.append(gb)

            # vsum columns (pre-scaled by -1/N), then vbneg assembly and
            # wneg = -(W~)^T : [12, 768]
            kva_row = single.tile([1, 768], bf16, tag="kva_row")
            nc.scalar.activation(kva_row[:], kvall_v[64:65, :], AF.Copy)
            vsum_ps = zpool.tile([128, 512], fp32, tag="zp", name="vsum_ps")[:, 0:12]
            for h in range(NUM_HEADS):
                wv_off = 64 * h - 64 * (h % 2)
                nc.tensor.matmul(vsum_ps[:, h:h + 1],
                                 kva_row[:, wv_off:wv_off + 128],
                                 negn_b[:, 0:1],
                                 start=True, stop=True, skip_group_check=True)
            for t in range(2):
                psl = slice(t * 64, (t + 1) * 64)
                nc.scalar.activation(
                    vb_flat[psl, t:t + 71:14], vsum_ps[psl, t:12:2], AF.Copy)
            for half in range(2):
                osl = slice(half * 384, (half + 1) * 384)
                psw = zpool.tile([128, 512], fp32, tag="zp", name=f"psw{half}")[0:NUM_HEADS, 0:384]
                for s in range(6):
                    nc.tensor.matmul(psw[:], vbneg[:, s, :], wo_s[:, s, osl],
                                     start=(s == 0), stop=(s == 5))
                nc.scalar.activation(wneg[:, osl], psw[:], AF.Copy)

            # ====== phase 2d: fold kv into the o_proj weights ======
            # per chunk: transpose the 2-head kv block, scale by SFAC on the
            # copy out, then W2_cc = kvT2_cc @ Wo rows for those heads
            for cc in range(6):
                pst = w2p.tile([128, 64], bf16, tag="w2t", name=f"pst{cc}")
                nc.tensor.transpose(pst[:], kvall_v[0:64, cc * 128:(cc + 1) * 128],
                                    ident_s[:])
                for t in range(2):
                    psl = slice(t * 64, (t + 1) * 64)
                    nc.vector.tensor_scalar_mul(kvT2[psl, cc, t * 64:t * 64 + 64],
                                                pst[psl, :], SFAC)
            for cc in range(6):
                for half in range(2):
                    osl = slice(half * 384, (half + 1) * 384)
                    psw2 = opool.tile([128, 512], fp32, tag="op",
                                      name=f"psw2_{cc}_{half}")[:, 0:384]
                    nc.tensor.matmul(psw2[:], kvT2[:, cc, :], wo_s[:, cc, osl],
                                     start=True, stop=True)
                    nc.vector.tensor_copy(w2_s[:, cc, osl], psw2[:])

            # ====== phase 3+4: per-slice G, q~g, o_proj (pipelined) ======
            for nq in range(NF):
                nsl = slice(nq * 512, (nq + 1) * 512)
                for cc in range(6):
                    psg = gpool.tile([128, 512], fp32, tag="gp", name=f"psg{nq}_{cc}")
                    nc.tensor.matmul(psg[:], eblk_s[:, cc, :], gbs[nq][:],
                                     start=True, stop=True)
                    nc.vector.tensor_tensor(qrope[:, cc, nsl], qrope[:, cc, nsl],
                                            psg[:], OP.mult)
                # o_proj for this n-slice: out = (q~g) @ W2 - z @ W~ + bo
                for c2 in range(6):
                    c2sl = slice(c2 * 128, (c2 + 1) * 128)
                    pso = opool.tile([128, 512], fp32, tag="op", name=f"pso{nq}_{c2}")
                    nc.tensor.matmul(pso[:], wneg[:, c2sl], zb[:, nsl],
                                     start=True, stop=False)
                    for s in range(6):
                        nc.tensor.matmul(pso[:], w2_s[:, s, c2sl], qrope[:, s, nsl],
                                         start=False, stop=(s == 5))
                    osb = work.tile([128, 512], fp16, tag="osb")
                    nc.scalar.activation(osb[:], pso[:], AF.Identity,
                                         bias=bo_s[:, c2:c2 + 1])
                    nc.sync.dma_start(outT[c2sl, nsl], osb[:])

            if DEBUG:
                nc.sync.dma_start(dbg_qrope[:], qrope[:])
                nc.sync.dma_start(dbg_zb[:], zb[:])
                nc.sync.dma_start(dbg_sums[:], sums_all[:])
                nc.sync.dma_start(dbg_w2[:], w2_s[:])
                nc.sync.dma_start(dbg_wneg[:], wneg[:])
                nc.sync.dma_start(dbg_kblock[:], kblock[:])

    nc.compile()
    return nc


def _get_nc():
    if "nc" not in _CACHE:
        _CACHE["nc"] = _build_nc()
    return _CACHE["nc"]


# --------------------------------------------------------------------------
# entry point
# --------------------------------------------------------------------------

def kernel(query, key, value, Wq, bq, Wk, bk, Wv, bv, Wo, bo, H, W):
    from concourse.bass_utils import run_bass_kernel_spmd

    assert int(H) == 64 and int(W) == 64
    query = np.asarray(query, np.float32)
    key = np.asarray(key, np.float32)
    value = np.asarray(value, np.float32)
    in_maps = _build_host_inputs(
        query, key, value,
        np.asarray(Wq, np.float32), np.asarray(bq, np.float32),
        np.asarray(Wk, np.float32), np.asarray(bk, np.float32),
        np.asarray(Wv, np.float32), np.asarray(bv, np.float32),
        np.asarray(Wo, np.float32), np.asarray(bo, np.float32))

    nc = _get_nc()
    kwargs = {}
    if os.environ.get("KERNEL_TRACE") == "1":
        kwargs = dict(trace=True, tmpdir=tempfile.mkdtemp(prefix="malat_"))
    r = run_bass_kernel_spmd(nc, in_maps, core_ids=list(range(NCORES)), **kwargs)
    LAST_RESULTS[0] = r

    out = np.empty((B, N, C), np.float32)
    for core in range(NCORES):
        b = core // 2
        sl = slice((core % 2) * NLOC, (core % 2 + 1) * NLOC)
        out[b, sl, :] = r.results[core]["outT"].T.astype(np.float32)
    return out


# revision 49
# speedup vs baseline: 1.4609x; 1.4609x over previous
"""MALA attention (linear attention w/ 2D RoPE + magnitude term) on 8 trn2 cores.

Sharding: core i handles batch b = i//2, sequence rows (i%2)*2048..+2048.
Cross-core data (kv = k_rope^T v, k_sum, v_sum -- all sums over the full
sequence) is combined with a pairwise AllReduce (~103KB bf16). Everything else
is local. bf16 operands everywhere with fp32 PSUM accumulation.

Math (per batch b, head h, reference semantics):
  q = elu(query @ Wq.T + bq) + 1      (same k with Wk/bk; v plain)
  z = (q . mean_n(k)) * d^-0.5
  q, k <- rope(q), rope(k)
  kv = k^T v * (d^-0.5 / N)
  res = (q @ kv) * (1 + 1/(z+1e-6)) - z * mean_n(v)
  out = res @ Wo.T + bo

Device-side formulation:
  elu(x)+1 = max(x + 1, min(exp(x), 1)) on the k path (DVE stt, bf16 ops);
  on the q path elu(u)+1 = min(exp(u),1) + relu(u): both exp and relu get
  the +bq bias via Act per-partition bias, so all q bias-opener matmuls are
  gone and the stt runs at bf16 rate.
  g = 1 + 1/(z+eps) folded into q_rope before o_proj (G built by one-hot
  matmul). kv is folded INTO the o_proj weights: W2 = (kv*SFAC) @ Wo.T per
  head (12 small matmuls post-CC, via a tensor-engine transpose of the kv
  block), so out = (q~g) @ W2 + z @ (-W~) + bo in a single accumulation --
  the separate qkv matmul and res round-trip are gone.
  -z*v_mean term folded into o_proj as an extra matmul (-W~)^T @ z where
  W~[c,h] = Wo[:, head h] @ v_mean_h.
  The within-head d-index is permuted (evens first) on Wq/Wk columns + trig
  tables so rotate_every_two becomes a 32-column block swap.
  Collective payload is bf16, pre-scaled by SFAC where possible: kv
  v-halves+vsum row [65,768] plus per-head ksums pre-transposed to columns
  [128,12]. Payload DMAs ride the otherwise-empty gpsimd queue right in
  front of the CC trigger; post-CC loads ride the parked sync queue, sums
  first, and kblock assembly is a strided sync-queue SBUF-SBUF DMA so the
  z chain doesn't queue behind phase-2a activations.
"""

import os
import tempfile

import numpy as np
import ml_dtypes

NUM_HEADS = 12
B, N, C = 4, 4096, 768
D = 64
NCORES = 8
NLOC = N // 2          # rows per core
SCALE = D ** -0.5
BF16 = ml_dtypes.bfloat16

_CACHE = {}
LAST_RESULTS = [None]  # test.py reads profiling info from here


# --------------------------------------------------------------------------
# host-side helpers
# --------------------------------------------------------------------------

def _perm64():
    # evens first, then odds (within each head's 64 dims)
    return np.concatenate([np.arange(0, 64, 2), np.arange(1, 64, 2)])


def _trig_tables():
    """c32/s32: [N, 32] fp32, value of cos/sin at original dim 2i (== 2i+1)."""
    H = W = 64
    angle = 1.0 / (10000.0 ** np.linspace(0.0, 1.0, D // 4))
    angle = np.repeat(angle, 2)                          # [32]
    ih = np.arange(H, dtype=np.float64)[:, None] * angle[None, :]   # [H, 32]
    iw = np.arange(W, dtype=np.float64)[:, None] * angle[None, :]
    sin_h, cos_h = np.sin(ih), np.cos(ih)
    sin_w, cos_w = np.sin(iw), np.cos(iw)
    r = np.arange(N) // W
    c = np.arange(N) % W
    s_full = np.concatenate([sin_h[r], sin_w[c]], axis=1)   # [N, 64]
    c_full = np.concatenate([cos_h[r], cos_w[c]], axis=1)
    c32 = c_full[:, 0::2].astype(np.float32)
    s32 = s_full[:, 0::2].astype(np.float32)
    return c32, s32


def _build_host_inputs(query, key, value, Wq, bq, Wk, bk, Wv, bv, Wo, bo):
    p64 = _perm64()
    perm = (np.arange(NUM_HEADS)[:, None] * 64 + p64[None, :]).reshape(-1)

    wq = np.ascontiguousarray(Wq.T[:, perm]).astype(BF16)
    wk = np.ascontiguousarray(Wk.T[:, perm]).astype(BF16)
    wv = np.ascontiguousarray(Wv.T).astype(BF16)
    wo = np.ascontiguousarray(Wo.T).astype(BF16)
    # q bias as per-partition columns for the Act bias add: [128, 6]
    bq_col = np.ascontiguousarray(
        bq[perm].astype(np.float32).reshape(6, 128).T)
    # k/v biases pre-broadcast across partitions for the DVE bias add
    bkp1_b = np.tile((bk[perm] + 1.0).astype(BF16)[None, :], (128, 1))
    bv_b = np.tile(bv.astype(BF16)[None, :], (128, 1))
    bof = bo.astype(np.float32)
    # identity for the kv transpose (values unused by the transpose datapath)
    ident = np.eye(64).astype(BF16)

    c32, s32 = _trig_tables()
    halves = []
    for hi in range(2):
        sl = slice(hi * NLOC, (hi + 1) * NLOC)
        ck = np.concatenate([c32[sl], c32[sl]], axis=1).astype(BF16)    # [NLOC, 64]
        s2k = np.concatenate([-s32[sl], s32[sl]], axis=1).astype(BF16)
        cq = np.tile(c32[sl].T, (4, 1)).astype(BF16)                    # [128, NLOC]
        s2q = np.tile(np.concatenate([-s32[sl].T, s32[sl].T], 0), (2, 1)).astype(BF16)
        halves.append((ck, s2k, cq, s2q))

    # one-hot G-broadcast lhsT: eblk[c][h, p] = 1 iff head h owns partition p
    # of chunk c (heads 2c: p<64, 2c+1: p>=64)
    eblk = np.zeros((6, NUM_HEADS, 128), dtype=BF16)
    for cc in range(6):
        eblk[cc, 2 * cc, :64] = 1.0
        eblk[cc, 2 * cc + 1, 64:] = 1.0

    in_maps = []
    for core in range(NCORES):
        b = core // 2
        hi = core % 2
        sl = slice(hi * NLOC, (hi + 1) * NLOC)
        ck, s2k, cq, s2q = halves[hi]
        in_maps.append({
            "xq": np.ascontiguousarray(query[b, sl].T).astype(BF16),
            "xk": np.ascontiguousarray(key[b, sl].T).astype(BF16),
            "xv": np.ascontiguousarray(value[b, sl].T).astype(BF16),
            "wq": wq, "wk": wk, "wv": wv, "wo": wo,
            "bq_col": bq_col, "bkp1_b": bkp1_b, "bv_b": bv_b, "bo": bof,
            "cos_k": ck, "s2_k": s2k, "cos_q": cq, "s2_q": s2q,
            "eblk": eblk, "ident": ident,
        })
    return in_maps


# --------------------------------------------------------------------------
# device kernel
# --------------------------------------------------------------------------

def _build_nc():
    import concourse.bacc as bacc
    import concourse.mybir as mybir
    import concourse.tile as tile

    fp32 = mybir.dt.float32
    bf16 = mybir.dt.bfloat16
    AF = mybir.ActivationFunctionType
    OP = mybir.AluOpType

    nc = bacc.Bacc("TRN2", target_bir_lowering=False, debug=False,
                   num_devices=NCORES)

    def din(name, shape, dt=bf16):
        return nc.dram_tensor(name, shape, dt, kind="ExternalInput").ap()

    NF = NLOC // 512       # 4 n-slices of 512
    NK = NLOC // 128       # 16 chunks of 128 for k/v phase

    xq = din("xq", [C, NLOC])
    xk = din("xk", [C, NLOC])
    xv = din("xv", [C, NLOC])
    wq_d = din("wq", [C, C])
    wk_d = din("wk", [C, C])
    wv_d = din("wv", [C, C])
    wo_d = din("wo", [C, C])
    bq_col_d = din("bq_col", [128, 6], fp32)
    bkp1_b_d = din("bkp1_b", [128, C])
    bv_b_d = din("bv_b", [128, C])
    bo_d = din("bo", [C], fp32)
    cos_k_d = din("cos_k", [NLOC, 64])
    s2_k_d = din("s2_k", [NLOC, 64])
    cos_q_d = din("cos_q", [128, NLOC])
    s2_q_d = din("s2_q", [128, NLOC])
    eblk_d = din("eblk", [6, NUM_HEADS, 128])
    ident_d = din("ident", [64, 64])
    fp16 = mybir.dt.float16
    outT = nc.dram_tensor("outT", [C, NLOC], fp16, kind="ExternalOutput").ap()
    DEBUG = os.environ.get("KERNEL_DEBUG") == "1"
    if DEBUG:
        dbg_qrope = nc.dram_tensor("dbg_qrope", [128, 6, NLOC], bf16, kind="ExternalOutput").ap()
        dbg_zb = nc.dram_tensor("dbg_zb", [NUM_HEADS, NLOC], bf16, kind="ExternalOutput").ap()
        dbg_sums = nc.dram_tensor("dbg_sums", [128, 12], bf16, kind="ExternalOutput").ap()
        dbg_w2 = nc.dram_tensor("dbg_w2", [128, 6, C], bf16, kind="ExternalOutput").ap()
        dbg_wneg = nc.dram_tensor("dbg_wneg", [NUM_HEADS, C], bf16, kind="ExternalOutput").ap()
        dbg_kblock = nc.dram_tensor("dbg_kblock", [128, 6, NUM_HEADS], bf16, kind="ExternalOutput").ap()

    SFAC = SCALE / N       # kv and z scale (2^-15, exact in bf16)
    NPAY = 65 * 768 + 128 * 12   # collective payload in bf16 elems

    with tile.TileContext(nc) as tc:
        from contextlib import ExitStack
        with ExitStack() as ctx:
            consts = ctx.enter_context(tc.tile_pool(name="consts", bufs=1))
            resid = ctx.enter_context(tc.tile_pool(name="resid", bufs=1))
            xin = ctx.enter_context(tc.tile_pool(name="xin", bufs=3))
            qx = ctx.enter_context(tc.tile_pool(name="qx", bufs=4))
            work = ctx.enter_context(tc.tile_pool(name="work", bufs=2))
            single = ctx.enter_context(tc.tile_pool(name="single", bufs=1))
            big = ctx.enter_context(tc.tile_pool(name="big", bufs=1))
            gsb = ctx.enter_context(tc.tile_pool(name="gsb", bufs=4))
            dram = ctx.enter_context(tc.tile_pool(name="dram", bufs=1, space="DRAM"))
            ph1 = ExitStack()
            pps = ph1.enter_context(tc.tile_pool(name="pps", bufs=4, space="PSUM"))
            kvp = ph1.enter_context(tc.tile_pool(name="kvp", bufs=1, space="PSUM"))

            # ---- phase-1-critical constants. xk0/xv0 ride the FRONT of the
            # scalar queue so they aren't stuck behind 2.4MB of weights on
            # the sync side ----
            xk0 = xin.tile([128, 6, 128], bf16, tag="xk_t", name="xk0")
            nc.scalar.dma_start(xk0[:], xk[:, 0:128].rearrange("(s p) n -> p s n", p=128))
            xv0 = xin.tile([128, 6, 128], bf16, tag="xv_t", name="xv0")
            nc.scalar.dma_start(xv0[:], xv[:, 0:128].rearrange("(s p) n -> p s n", p=128))
            wk_s = consts.tile([128, 6, C], bf16, tag="wk")
            for s in range(6):
                nc.sync.dma_start(wk_s[:, s, :],
                                  wk_d.rearrange("(s p) o -> p s o", p=128)[:, s, :])
            wv_s = consts.tile([128, 6, C], bf16, tag="wv")
            for s in range(6):
                nc.sync.dma_start(wv_s[:, s, :],
                                  wv_d.rearrange("(s p) o -> p s o", p=128)[:, s, :])
            bkb_s = consts.tile([128, C], bf16, tag="bkb")
            nc.scalar.dma_start(bkb_s[:], bkp1_b_d[:])
            bvb_s = consts.tile([128, C], bf16, tag="bvb")
            nc.scalar.dma_start(bvb_s[:], bv_b_d[:])
            cos_k_s = consts.tile([128, NK, 64], bf16, tag="cos_k")
            nc.scalar.dma_start(cos_k_s[:], cos_k_d.rearrange("(j p) d -> p j d", p=128))
            s2_k_s = consts.tile([128, NK, 64], bf16, tag="s2_k")
            nc.scalar.dma_start(s2_k_s[:], s2_k_d.rearrange("(j p) d -> p j d", p=128))
            xk1 = xin.tile([128, 6, 128], bf16, tag="xk_t", name="xk1")
            nc.sync.dma_start(xk1[:], xk[:, 128:256].rearrange("(s p) n -> p s n", p=128))
            xv1 = xin.tile([128, 6, 128], bf16, tag="xv_t", name="xv1")
            nc.sync.dma_start(xv1[:], xv[:, 128:256].rearrange("(s p) n -> p s n", p=128))
            # chunks 2-3 prefetch on the scalar queue too: they'd otherwise
            # queue behind 2.4MB of wk/wv on sync (measured ~4.7us of early
            # tensor stalls). xk3/xv3 reuse ring buffers (WAR on chunk-0
            # reads) but nothing latency-critical sits behind them here.
            xk2 = xin.tile([128, 6, 128], bf16, tag="xk_t", name="xk2")
            nc.scalar.dma_start(xk2[:], xk[:, 256:384].rearrange("(s p) n -> p s n", p=128))
            xv2 = xin.tile([128, 6, 128], bf16, tag="xv_t", name="xv2")
            nc.scalar.dma_start(xv2[:], xv[:, 256:384].rearrange("(s p) n -> p s n", p=128))
            bo_s = consts.tile([128, 6], fp32, tag="bo")
            nc.scalar.dma_start(bo_s[:], bo_d.rearrange("(s p) -> p s", p=128))
            bqc_s = consts.tile([128, 6], fp32, tag="bqc")
            nc.scalar.dma_start(bqc_s[:], bq_col_d[:])
            ident_s = consts.tile([64, 64], bf16, tag="ident")
            nc.scalar.dma_start(ident_s[:], ident_d[:])
            xk3 = xin.tile([128, 6, 128], bf16, tag="xk_t", name="xk3")
            nc.scalar.dma_start(xk3[:], xk[:, 384:512].rearrange("(s p) n -> p s n", p=128))
            xv3 = xin.tile([128, 6, 128], bf16, tag="xv_t", name="xv3")
            nc.scalar.dma_start(xv3[:], xv[:, 384:512].rearrange("(s p) n -> p s n", p=128))

            ones_row = consts.tile([1, 512], bf16, tag="ones_row")
            nc.vector.memset(ones_row[:], 1.0)
            negone = consts.tile([128, 1], fp32, tag="negone")
            nc.vector.memset(negone[:], -1.0)
            zero_col = consts.tile([1, 128], bf16, tag="zero_col")
            nc.vector.memset(zero_col[:], 0.0)
            # pre-scaled transpose constants: ksum cols arrive *SFAC,
            # vsum cols arrive *(-1/N) (both exact powers of two in bf16)
            sfac_b = consts.tile([1, 24], bf16, tag="sfac_b")
            nc.vector.memset(sfac_b[:], SFAC)
            negn_b = consts.tile([1, 24], bf16, tag="negn_b")
            nc.vector.memset(negn_b[:], -1.0 / N)

            # ---- persistent tiles ----
            qpre = big.tile([128, 6, NLOC], bf16, tag="qbig", name="qpre")
            qrope = resid.tile([128, 6, NLOC], bf16, tag="qrope")
            zb = resid.tile([NUM_HEADS, NLOC], bf16, tag="zb")
            kvT2 = resid.tile([128, 6, 128], bf16, tag="kvT2")
            nc.vector.memset(kvT2[:], 0.0)
            w2_s = consts.tile([128, 6, C], bf16, tag="w2")
            kblock = resid.tile([128, 6, NUM_HEADS], bf16, tag="kblock")
            nc.vector.memset(kblock[:], 0.0)
            vbneg = resid.tile([128, 6, NUM_HEADS], bf16, tag="vbneg")
            nc.vector.memset(vbneg[:], 0.0)
            wneg = resid.tile([NUM_HEADS, C], bf16, tag="wneg")

            # kv psums: 3 banks, persist through phase 1.
            # head h accumulates at [0:65, (h%4)*128 : +128] of tile h//4.
            # start=True clears the whole bank's has_written bits, so packing 4
            # heads' accumulation groups per bank needs a single bank-wide
            # zero-write group opener; all kv matmuls then accumulate.
            kvps = [kvp.tile([128, 512], fp32, tag=f"kvps{t}", name=f"kvps{t}")
                    for t in range(3)]
            for t in range(3):
                nc.tensor.matmul(kvps[t][0:65, :], zero_col[:, 0:65], ones_row[:],
                                 start=True, stop=False, skip_group_check=True)
            sums_ps = kvp.tile([128, 12], fp32, tag="sums_ps")

            # ================= phase 1: k/v proj, elu, rope, kv =================
            for j in range(NK):
                if j == 0:
                    xk_t, xv_t = xk0, xv0
                elif j == 1:
                    xk_t, xv_t = xk1, xv1
                elif j == 2:
                    xk_t, xv_t = xk2, xv2
                elif j == 3:
                    xk_t, xv_t = xk3, xv3
                else:
                    xk_t = xin.tile([128, 6, 128], bf16, tag="xk_t",
                                    name=f"xk{j}")
                    nc.sync.dma_start(
                        xk_t[:], xk[:, j * 128:(j + 1) * 128]
                        .rearrange("(s p) n -> p s n", p=128))
                    xv_t = xin.tile([128, 6, 128], bf16, tag="xv_t",
                                    name=f"xv{j}")
                    nc.sync.dma_start(
                        xv_t[:], xv[:, j * 128:(j + 1) * 128]
                        .rearrange("(s p) n -> p s n", p=128))

                vk = work.tile([128, NUM_HEADS, 128], bf16, tag="vk")
                e_t = work.tile([128, C], bf16, tag="e_t")
                tk = work.tile([128, C], bf16, tag="tk")
                kra = work.tile([128, NUM_HEADS, 66], bf16, tag="kra")
                nc.vector.memset(kra[:, :, 64:65], 1.0)

                # k projection (no bias opener; bias via DVE add)
                psk = [pps.tile([128, 384], fp32, tag="pp384", name=f"psk{half}")
                       for half in range(2)]
                for s in range(6):
                    for half in range(2):
                        nc.tensor.matmul(psk[half][:], xk_t[:, s, :],
                                         wk_s[:, s, half * 384:(half + 1) * 384],
                                         start=(s == 0), stop=(s == 5))
                for half in range(2):
                    osl = slice(half * 384, (half + 1) * 384)
                    hsl = slice(half * 6, (half + 1) * 6)
                    # tk = x + 1 (bias tile holds bk+1)
                    nc.vector.tensor_tensor(tk[:, osl], psk[half][:],
                                            bkb_s[:, osl], OP.add)
                    nc.scalar.activation(e_t[:, osl], tk[:, osl], AF.Exp,
                                         bias=negone[:])
                    nc.vector.scalar_tensor_tensor(
                        vk[:, hsl, 64:128],
                        e_t[:, osl].rearrange("p (h e) -> p h e", e=64),
                        1.0, tk[:, osl].rearrange("p (h e) -> p h e", e=64),
                        OP.min, OP.max)

                # v projection (bias via DVE add)
                psv = [pps.tile([128, 384], fp32, tag="pp384", name=f"psv{half}")
                       for half in range(2)]
                for s in range(6):
                    for half in range(2):
                        nc.tensor.matmul(psv[half][:], xv_t[:, s, :],
                                         wv_s[:, s, half * 384:(half + 1) * 384],
                                         start=(s == 0), stop=(s == 5))
                for half in range(2):
                    osl = slice(half * 384, (half + 1) * 384)
                    hsl = slice(half * 6, (half + 1) * 6)
                    nc.vector.tensor_tensor(
                        vk[:, hsl, 0:64],
                        psv[half][:].rearrange("p (h e) -> p h e", e=64),
                        bvb_s[:, osl].rearrange("p (h e) -> p h e", e=64),
                        OP.add)

                # rope on k_pre -> kra[:, :, 0:64]; s2 mults on gpsimd.
                # split per head-half so the kv matmuls of heads 0-5 fire
                # before heads 6-11's rope finishes (shorter chunk tail)
                tmpb = work.tile([128, NUM_HEADS, 64], bf16, tag="tmpb")
                for hh in range(2):
                    hsl2 = slice(hh * 6, (hh + 1) * 6)
                    cosj = cos_k_s[:, j, None, :].to_broadcast([128, 6, 64])
                    s2t = s2_k_s[:, j, None, 0:32].to_broadcast([128, 6, 32])
                    s2b = s2_k_s[:, j, None, 32:64].to_broadcast([128, 6, 32])
                    nc.vector.tensor_tensor(kra[:, hsl2, 0:64],
                                            vk[:, hsl2, 64:128], cosj, OP.mult)
                    nc.gpsimd.tensor_tensor(tmpb[:, hsl2, 0:32],
                                            vk[:, hsl2, 96:128], s2t, OP.mult)
                    nc.gpsimd.tensor_tensor(tmpb[:, hsl2, 32:64],
                                            vk[:, hsl2, 64:96], s2b, OP.mult)
                    nc.vector.tensor_tensor(kra[:, hsl2, 0:64],
                                            kra[:, hsl2, 0:64],
                                            tmpb[:, hsl2, :], OP.add)

                # kv accumulation: [k_rope | 1]^T @ [v | k_pre] per head
                for h in range(NUM_HEADS):
                    nc.tensor.matmul(
                        kvps[h // 4][0:65, (h % 4) * 128:(h % 4) * 128 + 128],
                        kra[:, h, 0:65], vk[:, h, :],
                        start=False, stop=(j == NK - 1), skip_group_check=True)

            # ====== phase 1.5: slim bf16 payload + pre-transposed ksums, CC
            # v-halves of kv psums (rows 0:65; row 64 = vsum) -> [65, 3, 256]
            # on DVE (Act handles the rows; parallel payload prep)
            kvsb_v = single.tile([65, 3, 256], bf16, tag="kvsb_v")
            for t in range(3):
                nc.vector.tensor_copy(
                    kvsb_v[:, t, :].rearrange("p (m e) -> p m e", e=64),
                    kvps[t][0:65, :].rearrange("p (m c) -> p m c", c=128)[:, :, 0:64])
            # full sums row (row 64: [vsum_h | ksum_h] per 128-col head block)
            kvs_row = single.tile([1, 1536], bf16, tag="kvs_row")
            for t in range(3):
                nc.scalar.activation(kvs_row[:, t * 512:(t + 1) * 512],
                                     kvps[t][64:65, :], AF.Copy)
            # transpose ksums to columns via bf16 K=1 matmuls (pre-scaled by
            # SFAC); col h = ksum_h*SFAC at partitions (h%2)*64 .. +64
            for h in range(NUM_HEADS):
                wk_off = 128 * h + 64 - 64 * (h % 2)
                nc.tensor.matmul(sums_ps[:, h:h + 1],
                                 kvs_row[:, wk_off:wk_off + 128],
                                 sfac_b[:, 0:1],
                                 start=True, stop=True, skip_group_check=True)
            sums_sb = single.tile([128, 12], bf16, tag="sums_sb")
            nc.scalar.activation(sums_sb[:], sums_ps[:], AF.Copy)

            # payload pushes on the gpsimd queue (empty, and the CC trigger
            # is the next gpsimd instruction -> no foreign descriptors ahead)
            bounce_in = dram.tile([NPAY], bf16, tag="b_in")
            bounce_out = dram.tile([NPAY], bf16, tag="b_out")
            nc.gpsimd.dma_start(
                bounce_in[0:65 * 768].rearrange("(p f) -> p f", p=65),
                kvsb_v.rearrange("p a b -> p (a b)"))
            nc.gpsimd.dma_start(
                bounce_in[65 * 768:NPAY].rearrange("(p f) -> p f", p=128),
                sums_sb[:])
            nc.gpsimd.collective_compute(
                "AllReduce", OP.add,
                replica_groups=[[0, 1], [2, 3], [4, 5], [6, 7]],
                ins=[bounce_in.opt()], outs=[bounce_out.opt()])

            # ---- late consts (needed from phase 2a on). xq slices ride in
            # front of / between the weights so slice-0 isn't stuck behind
            # 2.4MB of wq/wo ----
            xq_ts = [qx.tile([128, 6, 512], bf16, tag="xq_t", name=f"xq{nq}")
                     for nq in range(NF)]

            def xq_load(nq):
                nc.sync.dma_start(
                    xq_ts[nq][:], xq[:, nq * 512:(nq + 1) * 512]
                    .rearrange("(s p) n -> p s n", p=128))

            xq_load(0)
            wq_s = consts.tile([128, 6, C], bf16, tag="wq")
            nc.sync.dma_start(wq_s[:], wq_d.rearrange("(s p) o -> p s o", p=128))
            xq_load(1)
            cos_q_s = consts.tile([128, NLOC], bf16, tag="cos_q")
            nc.sync.dma_start(cos_q_s[:], cos_q_d[:])
            s2_q_s = consts.tile([128, NLOC], bf16, tag="s2_q")
            nc.sync.dma_start(s2_q_s[:], s2_q_d[:])
            for nq in range(2, NF):
                xq_load(nq)
            wo_s = consts.tile([128, 6, C], bf16, tag="wo")
            nc.sync.dma_start(wo_s[:], wo_d.rearrange("(s p) o -> p s o", p=128))
            eblk_s = consts.tile([NUM_HEADS, 6, 128], bf16, tag="eblk")
            nc.sync.dma_start(eblk_s[:], eblk_d.rearrange("c h p -> h c p"))

            # free phase-1 psum banks; phase 2a gets all 8 for psq
            ph1.close()
            ph2 = ExitStack()
            qps = ph2.enter_context(tc.tile_pool(name="qps", bufs=8, space="PSUM"))

            # ================= phase 2a: q proj, elu, rope =================
            for nq in range(NF):
                nsl = slice(nq * 512, (nq + 1) * 512)
                xq_t = xq_ts[nq]
                for oc in range(6):
                    psq = qps.tile([128, 512], fp32, tag="psq", name=f"psq{nq}_{oc}")
                    for s in range(6):
                        nc.tensor.matmul(psq[:], wq_s[:, s, oc * 128:(oc + 1) * 128],
                                         xq_t[:, s, :], start=(s == 0), stop=(s == 5))
                    # elu(u)+1 = min(exp(u),1) + relu(u); bias via Act bias ptr
                    e_q = work.tile([128, 512], bf16, tag="e_q")
                    nc.scalar.activation(e_q[:], psq[:], AF.Exp,
                                         bias=bqc_s[:, oc:oc + 1])
                    rx = work.tile([128, 512], bf16, tag="rx")
                    nc.scalar.activation(rx[:], psq[:], AF.Relu,
                                         bias=bqc_s[:, oc:oc + 1])
                    nc.vector.scalar_tensor_tensor(
                        qpre[:, oc, nsl], e_q[:], 1.0, rx[:], OP.min, OP.add)

                # rope: A + B with B reading the 32-block-swapped q_pre.
                # swap DMAs ride the gpsimd queue (sync is busy with consts)
                qsw = work.tile([128, 6, 512], bf16, tag="qsw")
                for g4 in range(4):
                    sp = (g4 ^ 1) * 32
                    nc.gpsimd.dma_start(qsw[g4 * 32:(g4 + 1) * 32, :, :],
                                        qpre[sp:sp + 32, :, nsl])
                for oc in range(6):
                    nc.vector.tensor_tensor(qrope[:, oc, nsl], qpre[:, oc, nsl],
                                            cos_q_s[:, nsl], OP.mult)
                    tmpq = work.tile([128, 512], bf16, tag="tmpq")
                    eng = nc.gpsimd if oc < 3 else nc.vector
                    eng.tensor_tensor(tmpq[:], qsw[:, oc, :], s2_q_s[:, nsl],
                                      OP.mult)
                    nc.vector.tensor_tensor(qrope[:, oc, nsl], qrope[:, oc, nsl],
                                            tmpq[:], OP.add)

            # ================= phase 2b: post-collective assembly =============
            # post-CC loads ride the SYNC queue: the sync engine has issued
            # everything else already and parks at the CC-wait. sums first --
            # they gate the z chain.
            sums_all = single.tile([128, 12], bf16, tag="sums_all")
            nc.sync.dma_start(sums_all[:],
                              bounce_out[65 * 768:NPAY].rearrange("(p f) -> p f", p=128))
            kvall_v = single.tile([65, 768], bf16, tag="kvall_v")
            nc.sync.dma_start(kvall_v[:],
                              bounce_out[0:65 * 768].rearrange("(p f) -> p f", p=65))

            # kblock (z weights; payload already *SFAC): strided SBUF-SBUF
            # DMAs on the parked sync queue -- no busy engine in the way
            kb_flat = kblock.rearrange("p s h -> p (s h)")
            vb_flat = vbneg.rearrange("p s h -> p (s h)")
            for t in range(2):
                psl = slice(t * 64, (t + 1) * 64)
                nc.sync.dma_start(kb_flat[psl, t:t + 71:14],
                                  sums_all[psl, t:12:2])

            ph2.close()
            zpool = ctx.enter_context(tc.tile_pool(name="zpool", bufs=2, space="PSUM"))
            gpool = ctx.enter_context(tc.tile_pool(name="gpool", bufs=2, space="PSUM"))
            w2p = ctx.enter_context(tc.tile_pool(name="w2p", bufs=2, space="PSUM"))
            opool = ctx.enter_context(tc.tile_pool(name="opool", bufs=2, space="PSUM"))

            # ========== phase 2c: z chains for all slices first ==========
            gbs = []
            for nq in range(NF):
                nsl = slice(nq * 512, (nq + 1) * 512)
                psz = zpool.tile([128, 512], fp32, tag="zp", name=f"psz{nq}")[0:NUM_HEADS, :]
                for s in range(6):
                    nc.tensor.matmul(psz[:], kblock[:, s, :], qpre[:, s, nsl],
                                     start=(s == 0), stop=(s == 5))
                gf = work.tile([NUM_HEADS, 512], fp32, tag="gf")
                nc.vector.reciprocal_approx_fast(gf[:], psz[:])  # z >= 7, eps moot
                gb = gsb.tile([NUM_HEADS, 512], bf16, tag="gb", name=f"gb{nq}")
                nc.vector.tensor_scalar_add(gb[:], gf[:], 1.0)
                nc.scalar.activation(zb[:, nsl], psz[:], AF.Copy)
                gbs.append(gb)

            # vsum columns (pre-scaled by -1/N), then vbneg assembly and
            # wneg = -(W~)^T : [12, 768]
            kva_row = single.tile([1, 768], bf16, tag="kva_row")
            nc.scalar.activation(kva_row[:], kvall_v[64:65, :], AF.Copy)
            vsum_ps = zpool.tile([128, 512], fp32, tag="zp", name="vsum_ps")[:, 0:12]
            for h in range(NUM_HEADS):
                wv_off = 64 * h - 64 * (h % 2)
                nc.tensor.matmul(vsum_ps[:, h:h + 1],
                                 kva_row[:, wv_off:wv_off + 128],
                                 negn_b[:, 0:1],
                                 start=True, stop=True, skip_group_check=True)
            for t in range(2):
                psl = slice(t * 64, (t + 1) * 64)
                nc.scalar.activation(
                    vb_flat[psl, t:t + 71:14], vsum_ps[psl, t:12:2], AF.Copy)
            for half in range(2):
                osl = slice(half * 384, (half + 1) * 384)
                psw = zpool.tile([128, 512], fp32, tag="zp", name=f"psw{half}")[0:NUM_HEADS, 0:384]
                for s in range(6):
                    nc.tensor.matmul(psw[:], vbneg[:, s, :], wo_s[:, s, osl],
                                     start=(s == 0), stop=(s == 5))
                nc.scalar.activation(wneg[:, osl], psw[:], AF.Copy)

            # ====== phase 2d: fold kv into the o_proj weights ======
            # per chunk: transpose the 2-head kv block, scale by SFAC on the
            # copy out, then W2_cc = kvT2_cc @ Wo rows for those heads
            for cc in range(6):
                pst = w2p.tile([128, 64], bf16, tag="w2t", name=f"pst{cc}")
                nc.tensor.transpose(pst[:], kvall_v[0:64, cc * 128:(cc + 1) * 128],
                                    ident_s[:])
                for t in range(2):
                    psl = slice(t * 64, (t + 1) * 64)
                    nc.vector.tensor_scalar_mul(kvT2[psl, cc, t * 64:t * 64 + 64],
                                                pst[psl, :], SFAC)
            for cc in range(6):
                for half in range(2):
                    osl = slice(half * 384, (half + 1) * 384)
                    psw2 = opool.tile([128, 512], fp32, tag="op",
                                      name=f"psw2_{cc}_{half}")[:, 0:384]
                    nc.tensor.matmul(psw2[:], kvT2[:, cc, :], wo_s[:, cc, osl],
                                     start=True, stop=True)
                    nc.vector.tensor_copy(w2_s[:, cc, osl], psw2[:])

            # ====== phase 3+4: per-slice G, q~g, o_proj (pipelined) ======
            for nq in range(NF):
                nsl = slice(nq * 512, (nq + 1) * 512)
                for cc in range(6):
                    psg = gpool.tile([128, 512], fp32, tag="gp", name=f"psg{nq}_{cc}")
                    nc.tensor.matmul(psg[:], eblk_s[:, cc, :], gbs[nq][:],
                                     start=True, stop=True)
                    nc.vector.tensor_tensor(qrope[:, cc, nsl], qrope[:, cc, nsl],
                                            psg[:], OP.mult)
                # o_proj for this n-slice: out = (q~g) @ W2 - z @ W~ + bo
                for c2 in range(6):
                    c2sl = slice(c2 * 128, (c2 + 1) * 128)
                    pso = opool.tile([128, 512], fp32, tag="op", name=f"pso{nq}_{c2}")
                    nc.tensor.matmul(pso[:], wneg[:, c2sl], zb[:, nsl],
                                     start=True, stop=False)
                    for s in range(6):
                        nc.tensor.matmul(pso[:], w2_s[:, s, c2sl], qrope[:, s, nsl],
                                         start=False, stop=(s == 5))
                    osb = work.tile([128, 512], fp16, tag="osb")
                    nc.scalar.activation(osb[:], pso[:], AF.Identity,
                                         bias=bo_s[:, c2:c2 + 1])
                    nc.sync.dma_start(outT[c2sl, nsl], osb[:])

            if DEBUG:
                nc.sync.dma_start(dbg_qrope[:], qrope[:])
                nc.sync.dma_start(dbg_zb[:], zb[:])
                nc.sync.dma_start(dbg_sums[:], sums_all[:])
                nc.sync.dma_start(dbg_w2[:], w2_s[:])
                nc.sync.dma_start(dbg_wneg[:], wneg[:])
                nc.sync.dma_start(dbg_kblock[:], kblock[:])

    nc.compile()
    return nc


def _get_nc():
    if "nc" not in _CACHE:
        _CACHE["nc"] = _build_nc()
    return _CACHE["nc"]


# --------------------------------------------------------------------------
# entry point
# --------------------------------------------------------------------------

def kernel(query, key, value, Wq, bq, Wk, bk, Wv, bv, Wo, bo, H, W):
    from concourse.bass_utils import run_bass_kernel_spmd

    assert int(H) == 64 and int(W) == 64
    query = np.asarray(query, np.float32)
    key = np.asarray(key, np.float32)
    value = np.asarray(value, np.float32)
    in_maps = _build_host_inputs(
        query, key, value,
        np.asarray(Wq, np.float32), np.asarray(bq, np.float32),
        np.asarray(Wk, np.float32), np.asarray(bk, np.float32),
        np.asarray(Wv, np.float32), np.asarray(bv, np.float32),
        np.asarray(Wo, np.float32), np.asarray(bo, np.float32))

    nc = _get_nc()
    kwargs = {}
    if os.environ.get("KERNEL_TRACE") == "1":
        kwargs = dict(trace=True, tmpdir=tempfile.mkdtemp(prefix="malat_"))
    r = run_bass_kernel_spmd(nc, in_maps, core_ids=list(range(NCORES)), **kwargs)
    LAST_RESULTS[0] = r

    out = np.empty((B, N, C), np.float32)
    for core in range(NCORES):
        b = core // 2
        sl = slice((core % 2) * NLOC, (core % 2 + 1) * NLOC)
        out[b, sl, :] = r.results[core]["outT"].T.astype(np.float32)
    return out
